# revision 18
# baseline (speedup 1.0000x reference)
"""TRN2 Bass kernel for nn_Encoder_Decoder_Quaternion.

Quaternion RNN encoder-decoder:
  enc: h_{t+1} = tanh(x_t @ WxB + bx + h_t @ WhB);  out_enc[t] = qmul(qnorm(h @ WlB + bl), x_t)
  dec: feeds its own prediction back as input (chaotic feedback loop).

Strategy:
  - Data-parallel over batch: B=512 -> 64 per core on 8 cores (no collectives).
  - Quaternion-linear == dense matmul with a block-structured "big" matrix,
    built host-side.  Hidden state kept on-chip in transposed [H, B] layout.
  - The decoder feedback loop amplifies errors ~1.45x/step, so the whole
    h-chain needs fp32-grade precision.  fp32 PE matmuls measure ~435ns each
    (unhidden 4-byte self weight-load), so instead every matmul in the h-chain
    uses an fp16 hi/lo 3-term split:  a@W ~= a16@W16 + alo@W16 + a16@Wlo
    (error ~2^-22), which runs at the fp16 streaming floor (~27ns/matmul).
  - Per step: 8 psum groups [128,64], each accumulating 24 Wh + 3 Wx matmuls,
    then ACT tanh (with bx bias) back to SBUF; DVE maintains (h16, hlo).
  - Encoder output path (not fed back) is plain fp16, batched per chunk of
    tc_e steps in component-major packed tiles.
"""
import os
import numpy as np
from contextlib import ExitStack

import concourse.bacc as bacc
import concourse.tile as tile
import concourse.mybir as mybir
from concourse import masks
from concourse.bass_utils import run_bass_kernel_spmd

f32, f16 = mybir.dt.float32, mybir.dt.float16
AF = mybir.ActivationFunctionType
ALU = mybir.AluOpType
AX = mybir.AxisListType

T_ENC, T_TGT, B, D, H = 100, 26, 512, 96, 1024
T_DEC = T_TGT - 1
NCORES = 8
BC = B // NCORES          # 64 batch per core
KB = H // 128             # 8 h-chunks
Q = D // 4                # 24 quaternions

_CACHE = {}


# ---------------------------------------------------------------- program ---

def _emit_qmul_norm(nc, sb_pool, pred_ap, x_ap, out_ap, F, dt_work, tag,
                    refine=False):
    """normalize(pred) then Hamilton-multiply with x, component-major layout.

    pred_ap / x_ap / out_ap: [P, 4*F] APs where component c occupies columns
    [c*F, (c+1)*F).  All emitted ops are <=3D.  refine=True adds one
    Newton-Raphson step to 1/sqrt (ACT Sqrt is only ~1e-6 accurate, which the
    chaotic decoder loop would amplify).
    """
    P = pred_ap.shape[0]

    def f4(ap):  # [P, 4, F]
        return ap.rearrange("p (c f) -> p c f", c=4)

    def bc(c, n):  # pred comp c broadcast over n slots: [P, n, F]
        return pred_ap[:, c * F:(c + 1) * F].unsqueeze(1) \
            .broadcast_to((P, n, F))

    sq = sb_pool.tile([P, 4 * F], dt_work, tag=f"{tag}_sq", name=f"{tag}_sq")
    nc.vector.tensor_tensor(sq[:], pred_ap, pred_ap, ALU.mult)
    s = sb_pool.tile([P, F], f32, tag=f"{tag}_s", name=f"{tag}_s")
    nc.vector.tensor_reduce(s[:], sq[:].rearrange("p (c f) -> p f c", c=4),
                            AX.X, ALU.add)
    nrm = sb_pool.tile([P, F], f32, tag=f"{tag}_nrm", name=f"{tag}_nrm")
    nc.scalar.sqrt(nrm[:], s[:])
    rn = sb_pool.tile([P, F], f32, tag=f"{tag}_rn", name=f"{tag}_rn")
    nc.vector.reciprocal(rn[:], nrm[:])
    if refine:
        # rn <- rn * (1.5 - 0.5 * s * rn^2)
        t1 = sb_pool.tile([P, F], f32, tag=f"{tag}_t1", name=f"{tag}_t1")
        nc.vector.tensor_tensor(t1[:], rn[:], rn[:], ALU.mult)
        t2 = sb_pool.tile([P, F], f32, tag=f"{tag}_t2", name=f"{tag}_t2")
        nc.vector.tensor_tensor(t2[:], s[:], t1[:], ALU.mult)
        t3 = sb_pool.tile([P, F], f32, tag=f"{tag}_t3", name=f"{tag}_t3")
        nc.vector.tensor_scalar(t3[:], t2[:], -0.5, 1.5, ALU.mult, ALU.add)
        rn2 = sb_pool.tile([P, F], f32, tag=f"{tag}_rn2", name=f"{tag}_rn2")
        nc.vector.tensor_tensor(rn2[:], rn[:], t3[:], ALU.mult)
        rn = rn2

    Pg = [sb_pool.tile([P, 4 * F], dt_work, tag=f"{tag}_P{g}", name=f"{tag}_P{g}")
          for g in range(4)]
    xv = f4(x_ap)

    def pv(g, sl):
        return f4(Pg[g][:])[:, sl, :]

    STT = nc.vector.scalar_tensor_tensor
    M = ALU.mult
    STT(f4(Pg[0][:]), bc(0, 4), 1.0, xv, M, M)
    STT(pv(1, slice(1, None, 2)), bc(1, 2), 1.0, xv[:, ::2, :], M, M)
    STT(pv(1, slice(0, None, 2)), bc(1, 2), -1.0, xv[:, 1::2, :], M, M)
    STT(pv(2, slice(1, 3)), bc(2, 2), 1.0, xv[:, ::-3, :], M, M)
    STT(pv(2, slice(0, None, 3)), bc(2, 2), -1.0, xv[:, 2:0:-1, :], M, M)
    STT(pv(3, slice(2, 4)), bc(3, 2), 1.0, xv[:, 1::-1, :], M, M)
    STT(pv(3, slice(0, 2)), bc(3, 2), -1.0, xv[:, 3:1:-1, :], M, M)

    s01 = sb_pool.tile([P, 4 * F], dt_work, tag=f"{tag}_s01", name=f"{tag}_s01")
    s23 = sb_pool.tile([P, 4 * F], dt_work, tag=f"{tag}_s23", name=f"{tag}_s23")
    ss = sb_pool.tile([P, 4 * F], dt_work, tag=f"{tag}_ss", name=f"{tag}_ss")
    nc.vector.tensor_add(s01[:], Pg[0][:], Pg[1][:])
    nc.vector.tensor_add(s23[:], Pg[2][:], Pg[3][:])
    nc.vector.tensor_add(ss[:], s01[:], s23[:])
    rn_b = rn[:].unsqueeze(1).broadcast_to((P, 4, F))
    nc.vector.tensor_tensor(f4(out_ap), f4(ss[:]), rn_b, ALU.mult)


def build_program(t_enc, t_dec, tc_e):
    dbg = set(os.environ.get("KDBG", "").split(","))
    assert t_enc % tc_e == 0 and tc_e % 2 == 0
    tc2 = tc_e // 2
    F1 = tc2 * Q              # packed per-parity time-block size per comp
    nc = bacc.Bacc("TRN2", target_bir_lowering=False, debug=False,
                   enable_asserts=False)

    def din(name, shape, dt):
        return nc.dram_tensor(name, shape, dt, kind="ExternalInput").ap()

    def dout(name, shape, dt):
        return nc.dram_tensor(name, shape, dt, kind="ExternalOutput").ap()

    inpT16 = din("inpT16", [t_enc, 96, BC], f16)   # hi part, [t, D, b]
    inpTlo = din("inpTlo", [t_enc, 96, BC], f16)   # lo part
    inp16 = din("inp16", [t_enc, BC, 96], f16)     # [t, b, D] for output path
    x0T16 = din("x0T16", [96, BC], f16)
    x0Tlo = din("x0Tlo", [96, BC], f16)
    x0b = din("x0b", [BC, 96], f32)
    Wh16 = din("Wh16", [H, H], f16)
    Whlo = din("Whlo", [H, H], f16)
    Wx16 = din("Wx16", [96, H], f16)
    Wxlo = din("Wxlo", [96, H], f16)
    Wl16 = din("Wl16", [H, 96], f16)
    Wllo = din("Wllo", [H, 96], f16)
    bx2d = din("bx2d", [128, KB], f32)
    bl32 = din("bl32", [BC, 96], f32)
    out_enc = dout("out_enc", [t_enc, BC, 96], f32)
    out_dec = dout("out_dec", [t_dec, BC, 96], f32)

    with tile.TileContext(nc) as tc, ExitStack() as ctx:
        const = ctx.enter_context(tc.tile_pool(name="const", bufs=1))
        chunk = ctx.enter_context(tc.tile_pool(name="chunk", bufs=2))
        ewp = ctx.enter_context(tc.tile_pool(name="ewp", bufs=1))
        hp = ctx.enter_context(tc.tile_pool(name="hp", bufs=2))
        decp = ctx.enter_context(tc.tile_pool(name="decp", bufs=2))
        rnn_ps = ctx.enter_context(tc.tile_pool(name="rnn_ps", bufs=4, space="PSUM"))
        pred_ps = ctx.enter_context(tc.tile_pool(name="pred_ps", bufs=2, space="PSUM"))
        tr_ps = ctx.enter_context(tc.tile_pool(name="tr_ps", bufs=2, space="PSUM"))

        # ---- constants
        Wh16SB = const.tile([128, KB * H], f16)    # kb-chunk at cols [kb*H, ...)
        WhloSB = const.tile([128, KB * H], f16)
        for kb in range(KB):
            nc.sync.dma_start(Wh16SB[:, kb * H:(kb + 1) * H],
                              Wh16[kb * 128:(kb + 1) * 128, :])
            nc.sync.dma_start(WhloSB[:, kb * H:(kb + 1) * H],
                              Whlo[kb * 128:(kb + 1) * 128, :])
        Wx16SB = const.tile([96, H], f16)
        WxloSB = const.tile([96, H], f16)
        nc.sync.dma_start(Wx16SB[:], Wx16[:])
        nc.sync.dma_start(WxloSB[:], Wxlo[:])
        Wl16SB = const.tile([128, KB * 96], f16)
        WlloSB = const.tile([128, KB * 96], f16)
        for kb in range(KB):
            nc.sync.dma_start(Wl16SB[:, kb * 96:(kb + 1) * 96],
                              Wl16[kb * 128:(kb + 1) * 128, :])
            nc.sync.dma_start(WlloSB[:, kb * 96:(kb + 1) * 96],
                              Wllo[kb * 128:(kb + 1) * 128, :])
        bxSB = const.tile([128, KB], f32)
        nc.sync.dma_start(bxSB[:], bx2d[:])
        bl32SB = const.tile([BC, 96], f32)
        nc.sync.dma_start(bl32SB[:], bl32[:])
        ident = const.tile([128, 128], f32)
        masks.make_identity(nc, ident[:])
        x0T16SB = const.tile([96, BC], f16)
        nc.sync.dma_start(x0T16SB[:], x0T16[:])
        x0TloSB = const.tile([96, BC], f16)
        nc.sync.dma_start(x0TloSB[:], x0Tlo[:])
        x0b_SB = const.tile([BC, 96], f32)
        nc.sync.dma_start(x0b_SB[:], x0b[:])

        def rnn_step(h16, hlo, xT16_ap, xTlo_ap, first, hT_new, h16_new, hlo_new):
            """One recurrence step: psum groups + tanh + (h16, hlo) refresh."""
            for mb in range(KB):
                psum = rnn_ps.tile([128, BC], f32, name="psum")
                if not first:
                    for kb in range(KB):
                        w16 = Wh16SB[:, kb * H + mb * 128:kb * H + (mb + 1) * 128]
                        wlo = WhloSB[:, kb * H + mb * 128:kb * H + (mb + 1) * 128]
                        h16s = h16[:, kb * BC:(kb + 1) * BC]
                        hlos = hlo[:, kb * BC:(kb + 1) * BC]
                        nc.tensor.matmul(psum[:], w16, h16s,
                                         start=(kb == 0), stop=False)
                        nc.tensor.matmul(psum[:], w16, hlos,
                                         start=False, stop=False)
                        nc.tensor.matmul(psum[:], wlo, h16s,
                                         start=False, stop=False)
                wx16 = Wx16SB[:, mb * 128:(mb + 1) * 128]
                wxlo = WxloSB[:, mb * 128:(mb + 1) * 128]
                nc.tensor.matmul(psum[:], wx16, xT16_ap, start=first, stop=False)
                nc.tensor.matmul(psum[:], wx16, xTlo_ap, start=False, stop=False)
                nc.tensor.matmul(psum[:], wxlo, xT16_ap, start=False, stop=True)
                sl = slice(mb * BC, (mb + 1) * BC)
                nc.scalar.activation(hT_new[:, sl], psum[:], AF.Tanh,
                                     bias=bxSB[:, mb:mb + 1])
                nc.vector.tensor_copy(h16_new[:, sl], hT_new[:, sl])
                nc.vector.tensor_sub(hlo_new[:, sl], hT_new[:, sl], h16_new[:, sl])

        # ---- encoder
        h16 = hlo = None
        xT16_ch = xTlo_ch = x16_ch = pred16_ch = None
        for t in range(t_enc):
            j = t % tc_e
            if j == 0:
                xT16_ch = chunk.tile([96, tc_e * BC], f16, tag="xT16")
                xTlo_ch = chunk.tile([96, tc_e * BC], f16, tag="xTlo")
                for dst_t, src_t in ((xT16_ch, inpT16), (xTlo_ch, inpTlo)):
                    src = src_t[t:t + tc_e].rearrange("t p b -> p t b")
                    dst = dst_t[:].rearrange("p (t b) -> p t b", t=tc_e)
                    nc.sync.dma_start(dst, src)
                x16_ch = chunk.tile([128, 4 * F1], f16, tag="x16")
                pred16_ch = chunk.tile([128, 4 * F1], f16, tag="pred16")
                for par in range(2):
                    for c in range(4):
                        src = inp16[t + par:t + tc_e:2, :, c * Q:(c + 1) * Q] \
                            .rearrange("t p q -> p t q")
                        dst = x16_ch[par * 64:par * 64 + 64,
                                     c * F1:(c + 1) * F1] \
                            .rearrange("p (i q) -> p i q", i=tc2)
                        nc.sync.dma_start(dst, src)

            hT_new = hp.tile([128, KB * BC], f32, tag="hT", name="hT")
            h16_new = hp.tile([128, KB * BC], f16, tag="h16", name="h16")
            hlo_new = hp.tile([128, KB * BC], f16, tag="hlo", name="hlo")
            rnn_step(h16, hlo,
                     xT16_ch[:, j * BC:(j + 1) * BC],
                     xTlo_ch[:, j * BC:(j + 1) * BC],
                     t == 0, hT_new, h16_new, hlo_new)
            h16, hlo = h16_new, hlo_new

            # output projection (plain fp16)
            if "nopred" not in dbg:
                psp = pred_ps.tile([BC, 96], f32, tag="psp", name="psp")
                for kb in range(KB):
                    nc.tensor.matmul(psp[:], h16[:, kb * BC:(kb + 1) * BC],
                                     Wl16SB[:, kb * 96:(kb + 1) * 96],
                                     start=(kb == 0), stop=(kb == KB - 1))
                par, i = t % 2, j // 2
                dst = pred16_ch[par * 64:par * 64 + 64, :] \
                    .rearrange("p (c i q) -> p c i q", c=4, i=tc2)[:, :, i, :]
                nc.vector.scalar_tensor_tensor(
                    dst, psp[:].rearrange("p (c q) -> p c q", c=4), 1.0,
                    bl32SB[:].rearrange("p (c q) -> p c q", c=4),
                    ALU.mult, ALU.add)

            if j == tc_e - 1:
                out32 = chunk.tile([128, 4 * F1], f32, tag="out32")
                if "noew" in dbg or "nopred" in dbg:
                    nc.vector.tensor_copy(out32[:], x16_ch[:])
                else:
                    _emit_qmul_norm(nc, ewp, pred16_ch[:], x16_ch[:], out32[:],
                                    F1, f16, "enc")
                t0 = t - tc_e + 1
                for par in range(2):
                    for c in range(4):
                        src = out32[par * 64:par * 64 + 64,
                                    c * F1:(c + 1) * F1] \
                            .rearrange("p (i q) -> p i q", i=tc2)
                        dst = out_enc[t0 + par:t0 + tc_e:2, :,
                                      c * Q:(c + 1) * Q] \
                            .rearrange("t p q -> p t q")
                        nc.sync.dma_start(dst, src)

        # ---- decoder (split-precision h-chain + output feedback)
        xT16_cur, xTlo_cur, xb_cur = x0T16SB, x0TloSB, x0b_SB
        for t in range([] and t_dec or 0 if "nodec" in dbg else t_dec):
            pass
        if "nodec" in dbg:
            for t in range(t_dec):
                nc.sync.dma_start(out_dec[t], x0b_SB[:])
        for t in range(0 if "nodec" in dbg else t_dec):
            hT_new = hp.tile([128, KB * BC], f32, tag="hT", name="hTd")
            h16_new = hp.tile([128, KB * BC], f16, tag="h16", name="h16d")
            hlo_new = hp.tile([128, KB * BC], f16, tag="hlo", name="hlod")
            rnn_step(h16, hlo, xT16_cur[:], xTlo_cur[:], False,
                     hT_new, h16_new, hlo_new)
            h16, hlo = h16_new, hlo_new

            # prediction: split-precision h @ WlB + bl
            psp = pred_ps.tile([BC, 96], f32, tag="psp", name="pspd")
            for kb in range(KB):
                h16s = h16[:, kb * BC:(kb + 1) * BC]
                hlos = hlo[:, kb * BC:(kb + 1) * BC]
                wl16 = Wl16SB[:, kb * 96:(kb + 1) * 96]
                wllo = WlloSB[:, kb * 96:(kb + 1) * 96]
                nc.tensor.matmul(psp[:], h16s, wl16, start=(kb == 0), stop=False)
                nc.tensor.matmul(psp[:], h16s, wllo, start=False, stop=False)
                nc.tensor.matmul(psp[:], hlos, wl16, start=False,
                                 stop=(kb == KB - 1))
            predB = decp.tile([BC, 96], f32, tag="predB", name="predB")
            nc.vector.scalar_tensor_tensor(
                predB[:], psp[:], 1.0, bl32SB[:], ALU.mult, ALU.add)

            out_x = decp.tile([BC, 96], f32, tag="out_x", name="out_x")
            _emit_qmul_norm(nc, decp, predB[:], xb_cur[:], out_x[:], Q, f32,
                            "dec", refine=True)
            nc.sync.dma_start(out_dec[t], out_x[:])

            trp = tr_ps.tile([96, BC], f32, name="trp")
            nc.tensor.transpose(trp[:], out_x[:], ident[:BC, :BC])
            xT_new = decp.tile([96, BC], f32, tag="xT_dec", name="xT_dec")
            nc.scalar.copy(xT_new[:], trp[:])
            xT16_new = decp.tile([96, BC], f16, tag="xT16_dec", name="xT16_dec")
            nc.vector.tensor_copy(xT16_new[:], xT_new[:])
            xTlo_new = decp.tile([96, BC], f16, tag="xTlo_dec", name="xTlo_dec")
            nc.vector.tensor_sub(xTlo_new[:], xT_new[:], xT16_new[:])
            xT16_cur, xTlo_cur, xb_cur = xT16_new, xTlo_new, out_x

    nc.compile()
    return nc


# ------------------------------------------------------------------- host ---

def _build_big(W):
    """W [4, nin, nout] -> [4*nin, 4*nout] block matrix s.t. x @ big == qlinear."""
    Wr, Wi, Wj, Wk = W[0], W[1], W[2], W[3]
    return np.concatenate([
        np.concatenate([Wr, Wi, Wj, Wk], 1),
        np.concatenate([-Wi, Wr, -Wk, Wj], 1),
        np.concatenate([-Wj, Wk, Wr, -Wi], 1),
        np.concatenate([-Wk, -Wj, Wi, Wr], 1),
    ], 0).astype(np.float32)


def _split16(a):
    """fp32 array -> (hi_fp16, lo_fp16) with hi + lo ~= a to ~2^-22."""
    hi = a.astype(np.float16)
    lo = (a - hi.astype(np.float32)).astype(np.float16)
    return hi, lo


def _get_program(t_enc, t_dec, tc_e):
    key = (t_enc, t_dec, tc_e, os.environ.get("KDBG", ""))
    if key not in _CACHE:
        _CACHE[key] = build_program(t_enc, t_dec, tc_e)
    return _CACHE[key]


def prep_in_maps(inp, target, Wx, bx, Wh, Wl, bl, t_enc):
    WhB = _build_big(np.asarray(Wh))
    WxB = _build_big(np.asarray(Wx))
    WlB = _build_big(np.asarray(Wl))
    bx = np.asarray(bx, np.float32)
    bl = np.asarray(bl, np.float32)
    inp = np.asarray(inp, np.float32)
    target = np.asarray(target, np.float32)

    Wh16, Whlo = _split16(WhB)
    Wx16, Wxlo = _split16(WxB)
    Wl16, Wllo = _split16(WlB)

    common = dict(
        Wh16=Wh16, Whlo=Whlo, Wx16=Wx16, Wxlo=Wxlo, Wl16=Wl16, Wllo=Wllo,
        bx2d=np.ascontiguousarray(bx.reshape(KB, 128).T),
        bl32=np.ascontiguousarray(np.broadcast_to(bl.reshape(1, 96), (BC, 96))),
    )
    in_maps = []
    for c in range(NCORES):
        sl = slice(c * BC, (c + 1) * BC)
        inpT = np.ascontiguousarray(inp[:t_enc, sl, :].transpose(0, 2, 1))
        inpT16, inpTlo = _split16(inpT)
        x0T = np.ascontiguousarray(target[0, sl, :].T)
        x0T16, x0Tlo = _split16(x0T)
        in_maps.append(dict(
            common,
            inpT16=inpT16, inpTlo=inpTlo,
            inp16=np.ascontiguousarray(inp[:t_enc, sl, :]).astype(np.float16),
            x0T16=x0T16, x0Tlo=x0Tlo,
            x0b=np.ascontiguousarray(target[0, sl, :]),
        ))
    return in_maps


def run(inp, target, Wx, bx, Wh, Wl, bl, t_enc=None, t_dec=None, tc_e=20,
        trace=False):
    t_enc = t_enc or inp.shape[0]
    t_dec = t_dec if t_dec is not None else target.shape[0] - 1
    nc = _get_program(t_enc, t_dec, tc_e)
    in_maps = prep_in_maps(inp, target, Wx, bx, Wh, Wl, bl, t_enc)
    res = run_bass_kernel_spmd(nc, in_maps, core_ids=list(range(NCORES)),
                               trace=trace)
    enc = np.concatenate([r["out_enc"] for r in res.results], axis=1)
    dec = np.concatenate([r["out_dec"] for r in res.results], axis=1)
    return (enc, dec), res


def kernel(inp, target, Wx, bx, Wh, Wl, bl):
    (enc, dec), _ = run(inp, target, Wx, bx, Wh, Wl, bl)
    return enc, dec
